# revision 26
# baseline (speedup 1.0000x reference)
"""Trainium2 Bass kernel for nn_Net_18262200943034 (stereo cost-volume soft-argmin).

Math (validated vs reference at 7e-7 rel err):
  vol[b,d,h,w] = [w>=d] * (SL[b,h,w] + SR[b,h,w-d]),  SL/SR = channel-means
  out = soft-argmin over d' of trilinear-x4-upsampled vol  -> [B, 4H, 4W]

Per core (8 cores = batch 2 x four 64-row h' blocks):
  1. fused C-mean + H-interp as matmuls -> SLH^T [128w, 64h'], SRH [64h', 128w]
  2. Toeplitz DMA from zero-padded SRH in DRAM -> masked shifted term for all (h',d)
  3. W-upsample and D-upsample as matmuls (interp matrices as inputs)
  4. exp on ACT (values bounded, no max-subtract needed), softmax + soft-argmin
     reductions on DVE/GpSimd in big batched ops.

bf16 data path (PSUM accumulation stays fp32): avoids the fp32 HI/LO matmul
split, enables FWL weight loads and DVE 2x. All per-core inputs travel in ONE
[128, 2736] "mega" tensor so a single DMA semaphore covers every constant.
"""
import os
import numpy as np
import ml_dtypes

import concourse.bacc as bacc
import concourse.bass as bass
import concourse.mybir as mybir
import concourse.tile as tile
from concourse.bass_utils import run_bass_kernel_spmd

F32 = mybir.dt.float32
BF16 = mybir.dt.bfloat16
NPBF = ml_dtypes.bfloat16

B, C, H, W = 2, 32, 64, 128
D, DP = 48, 192
H4, W4 = 256, 512
HB = 64            # h' rows per core
HS = 18            # source h rows needed
HPAD = 20          # padded so C*HPAD = 640 = 5*128
KCH = 5            # K chunks of 128 for the (c,h) contraction
H_START = [0, 15, 31, 47]

# mega layout (free-dim offsets, bf16 elements)
OFF_LP, OFF_AT, OFF_RP = 0, 640, 960
OFF_MASK, OFF_V = 1600, 1648
OFF_UHI, OFF_ULO, OFF_ODHI, OFF_ODLO = 2160, 2288, 2352, 2384
MEGA_F = 2416


def _interp_pairs(in_size, out_size):
    src = (np.arange(out_size, dtype=np.float32)
           * np.float32((in_size - 1) / (out_size - 1)))
    i0 = np.clip(np.floor(src).astype(np.int32), 0, in_size - 1)
    i1 = np.clip(i0 + 1, 0, in_size - 1)
    w = (src - i0.astype(np.float32)).astype(np.float32)
    return i0, i1, w


def _interp_matrix(in_size, out_size):
    i0, i1, w = _interp_pairs(in_size, out_size)
    M = np.zeros((in_size, out_size), dtype=np.float32)
    for o in range(out_size):
        M[i0[o], o] += np.float32(1.0) - w[o]
        M[i1[o], o] += w[o]
    return M


def _shared_mega():
    """The core-independent part of the mega input."""
    mega = np.zeros((128, MEGA_F), np.float32)
    mega[:, OFF_MASK:OFF_MASK + D] = (
        np.arange(W)[:, None] >= (D - 1 - np.arange(D))[None, :]).astype(np.float32)
    mega[:, OFF_V:OFF_V + W4] = _interp_matrix(W, W4)
    U_rev = _interp_matrix(D, DP)[::-1]
    mega[0:48, OFF_UHI:OFF_UHI + 128] = U_rev[:, 0:128]
    mega[0:48, OFF_ULO:OFF_ULO + 64] = U_rev[:, 128:DP]
    mega[:, OFF_ODHI] = 1.0
    mega[:, OFF_ODHI + 1] = np.arange(128, dtype=np.float32)
    mega[0:64, OFF_ODLO] = 1.0
    mega[0:64, OFF_ODLO + 1] = np.arange(128, DP, dtype=np.float32)
    return mega  # od blocks are 32 wide; cols 2..31 stay zero


def _core_mega(shared, left, right, b, j):
    hs = H_START[j]
    nvalid = min(H, hs + HS) - hs
    lp = np.zeros((C, HPAD, W), np.float32)
    rp = np.zeros((C, HPAD, W), np.float32)
    lp[:, :nvalid] = left[b, :, hs:hs + nvalid]
    rp[:, :nvalid] = right[b, :, hs:hs + nvalid]

    i0, i1, w = _interp_pairs(H, H4)
    A = np.zeros((HB, HPAD), np.float32)
    inv2c = np.float32(1.0 / (2 * C))
    for i in range(HB):
        hp = HB * j + i
        A[i, i0[hp] - hs] += (np.float32(1.0) - w[hp]) * inv2c
        A[i, i1[hp] - hs] += w[hp] * inv2c
    aT = np.ascontiguousarray(
        np.broadcast_to(A.T[None], (C, HPAD, HB))).reshape(KCH, 128, HB)

    mega = shared.copy()
    # [(k p), x] -> [p, (k x)]
    mega[:, OFF_LP:OFF_LP + 640] = (
        lp.reshape(KCH, 128, W).transpose(1, 0, 2).reshape(128, KCH * W))
    mega[:, OFF_RP:OFF_RP + 640] = (
        rp.reshape(KCH, 128, W).transpose(1, 0, 2).reshape(128, KCH * W))
    mega[:, OFF_AT:OFF_AT + 320] = (
        aT.transpose(1, 0, 2).reshape(128, KCH * HB))
    return mega.astype(NPBF)


def build_nc():
    nc = bacc.Bacc("TRN2", target_bir_lowering=False, debug=False)

    mega_d = nc.declare_dram_parameter("mega", [128, MEGA_F], BF16, isOutput=False)
    outt_d = nc.declare_dram_parameter("outt", [HB, W4], F32, isOutput=True)
    srhp_dram = nc.dram_tensor("srhp", [HB, D + W], BF16)  # zero-padded SRH
    zd_dram = nc.dram_tensor("zd", [2, 16384], F32)        # Z|N reshape staging

    EXP = mybir.ActivationFunctionType.Exp
    AX = mybir.AxisListType.X

    with tile.TileContext(nc) as tc:
        with tc.tile_pool(name="consts", bufs=1) as cpool:
            mega_sb = cpool.tile([128, MEGA_F], BF16)
            nc.sync.dma_start(mega_sb[:], mega_d[:])

            lp_v = mega_sb[:, OFF_LP:OFF_LP + 640].rearrange("p (k w) -> p k w", k=KCH)
            rp_v = mega_sb[:, OFF_RP:OFF_RP + 640].rearrange("p (k w) -> p k w", k=KCH)
            aT_v = mega_sb[:, OFF_AT:OFF_AT + 320].rearrange("p (k m) -> p k m", k=KCH)
            mask_v = mega_sb[:, OFF_MASK:OFF_MASK + D]
            v_v = mega_sb[:, OFF_V:OFF_V + W4]
            uhi_v = mega_sb[0:48, OFF_UHI:OFF_UHI + 128]
            ulo_v = mega_sb[0:48, OFF_ULO:OFF_ULO + 64]
            odhi_v = mega_sb[:, OFF_ODHI:OFF_ODHI + 2]
            odlo_v = mega_sb[0:64, OFF_ODLO:OFF_ODLO + 2]

            # Stage A: SLH^T = lp^T @ aT  (contract (c,h)),  SRH = aT^T @ rp
            slht_sb = cpool.tile([W, HB], BF16)
            srhp_sb = cpool.tile([HB, D + W], BF16)
            with tc.tile_pool(name="psA", bufs=1, space="PSUM") as psA:
                slht_ps = psA.tile([W, HB], F32)
                srh_ps = psA.tile([HB, W], F32)
                for k in range(KCH):
                    nc.tensor.matmul(slht_ps[:], lp_v[:, k, :], aT_v[:, k, :],
                                     start=(k == 0), stop=(k == KCH - 1))
                for k in range(KCH):
                    nc.tensor.matmul(srh_ps[:], aT_v[:, k, :], rp_v[:, k, :],
                                     start=(k == 0), stop=(k == KCH - 1))

                nc.vector.tensor_copy(slht_sb[:], slht_ps[:])
                nc.vector.memset(srhp_sb[:, 0:D], 0.0)
                nc.vector.tensor_copy(srhp_sb[:, D:D + W], srh_ps[:])
            nc.sync.dma_start(srhp_dram[:], srhp_sb[:])

            # m2[w, h', dr] = maskT[w,dr] * (SLH^T[w,h'] + SRH_pad[h', w+dr+1])
            g_sb = cpool.tile([48, HB, W4], BF16)  # [dr, h', w']
            with tc.tile_pool(name="mwork", bufs=1) as mpool:
                toep_sb = mpool.tile([W, HB, D], BF16)
                toep_src = bass.AP(srhp_dram, 1, [[1, W], [D + W, HB], [1, D]])
                nc.sync.dma_start(toep_sb[:], toep_src)

                m_sb = mpool.tile([W, HB, D], BF16)
                slht_b = slht_sb[:].unsqueeze(2).broadcast_to((W, HB, D))
                nc.vector.tensor_add(m_sb[:], toep_sb[:], slht_b)
                m2_sb = mpool.tile([W, HB, D], BF16)
                mask_b = mask_v.unsqueeze(1).broadcast_to((W, HB, D))
                nc.vector.tensor_mul(m2_sb[:], m_sb[:], mask_b)

                # W-upsample: g[dr, h', w'] = sum_w m2[w, h', dr] V[w, w']
                with tc.tile_pool(name="psG", bufs=3, space="PSUM") as psG:
                    for p in range(32):
                        g_ps = psG.tile([48, 2, W4], F32)
                        for u in range(2):
                            nc.tensor.matmul(g_ps[:, u, :],
                                             m2_sb[:, 2 * p + u, :], v_v,
                                             start=True, stop=True)
                        nc.vector.tensor_copy(g_sb[:, 2 * p:2 * p + 2, :], g_ps[:])

            # D-up -> f^T [d'-part, (h',w')], exp in place, then Z and the
            # soft-argmin numerator via PE matmuls with [ones; d'] stationary.
            with (
                tc.tile_pool(name="epool", bufs=1) as epool,
                tc.tile_pool(name="spool", bufs=2) as spool,
            ):
                for sb in range(2):  # superblocks of 32 h'
                    e_hi = epool.tile([128, 32, W4], BF16, tag="ehi")
                    e_lo = epool.tile([64, 32, W4], BF16, tag="elo")
                    with (
                        tc.tile_pool(name="psF", bufs=2, space="PSUM") as psFh,
                        tc.tile_pool(name="psL", bufs=2, space="PSUM") as psFl,
                    ):
                        for t in range(16):  # pairs of h'
                            f_hi = psFh.tile([128, 2, W4], F32, tag="fh")
                            f_lo = psFl.tile([64, 2, W4], F32, tag="fl")
                            for u in range(2):
                                hp = 32 * sb + 2 * t + u
                                nc.tensor.matmul(f_hi[:, u, :], uhi_v,
                                                 g_sb[:, hp, :],
                                                 start=True, stop=True)
                                nc.tensor.matmul(f_lo[:, u, :], ulo_v,
                                                 g_sb[:, hp, :],
                                                 start=True, stop=True)
                            nc.scalar.activation(e_hi[:, 2 * t:2 * t + 2, :],
                                                 f_hi[:], EXP)
                            nc.scalar.activation(e_lo[:, 2 * t:2 * t + 2, :],
                                                 f_lo[:], EXP)

                    # Z/N -> [2, 16384] PSUM, long-copy out, DMA reshape dense
                    ehf = e_hi[:].rearrange("p a w -> p (a w)")
                    elf = e_lo[:].rearrange("p a w -> p (a w)")
                    with tc.tile_pool(name="psZ", bufs=2, space="PSUM") as psZ:
                        for gt in range(8):
                            znb = psZ.tile([2, 2048], F32, tag="zn")
                            for j in range(4):
                                s = 512 * (4 * gt + j)
                                nc.tensor.matmul(znb[:, 512 * j:512 * (j + 1)],
                                                 odhi_v, ehf[:, s:s + 512],
                                                 start=True, stop=False)
                                nc.tensor.matmul(znb[:, 512 * j:512 * (j + 1)],
                                                 odlo_v, elf[:, s:s + 512],
                                                 start=False, stop=True)
                            zsb_t = spool.tile([2, 2048], F32, tag="zsb")
                            nc.vector.tensor_copy(zsb_t[:], znb[:])
                            nc.sync.dma_start(
                                bass.AP(zd_dram, 2048 * gt,
                                        [[16384, 2], [1, 2048]]), zsb_t[:])
                    zc_t = spool.tile([128, 2, 128], F32, tag="zc")
                    nc.sync.dma_start(
                        zc_t[:], bass.AP(zd_dram, 0,
                                         [[1, 128], [16384, 2], [128, 128]]))
                    rz_t = spool.tile([128, 128], F32, tag="rz")
                    nc.vector.reciprocal(rz_t[:], zc_t[:, 0, :])
                    oc_t = spool.tile([128, 128], F32, tag="oc")
                    nc.vector.tensor_mul(oc_t[:], zc_t[:, 1, :], rz_t[:])
                    nc.sync.dma_start(
                        bass.AP(outt_d, 16384 * sb, [[1, 128], [128, 128]]),
                        oc_t[:])
    nc.compile()
    return nc


_NC = None


def _in_maps(left, right):
    shared = _shared_mega()
    return [{"mega": _core_mega(shared, left, right, k // 4, k % 4)}
            for k in range(8)]


def kernel(left, right):
    global _NC
    left = np.asarray(left, dtype=np.float32)
    right = np.asarray(right, dtype=np.float32)
    if _NC is None:
        _NC = build_nc()

    res = run_bass_kernel_spmd(_NC, _in_maps(left, right), core_ids=list(range(8)))
    out = np.zeros((B, H4, W4), np.float32)
    for k in range(8):
        b, j = k // 4, k % 4
        out[b, HB * j:HB * (j + 1)] = res.results[k]["outt"]
    return out


# revision 35
# speedup vs baseline: 1.2871x; 1.2871x over previous
"""Trainium2 Bass kernel for nn_Net_18262200943034 (stereo cost-volume soft-argmin).

Math (validated vs reference at 7e-7 rel err):
  vol[b,d,h,w] = [w>=d] * (SL[b,h,w] + SR[b,h,w-d]),  SL/SR = channel-means
  out = soft-argmin over d' of trilinear-x4-upsampled vol  -> [B, 4H, 4W]

Per core (8 cores = batch 2 x four 64-row h' blocks):
  1. fused C-mean + H-interp as matmuls -> SLH^T [128w, 64h'], SRH [64h', 128w]
  2. Toeplitz DMA from zero-padded SRH in DRAM -> masked shifted term for all (h',d)
  3. W-upsample and D-upsample as matmuls (interp matrices as inputs)
  4. exp on ACT (values bounded, no max-subtract needed), softmax + soft-argmin
     reductions on DVE/GpSimd in big batched ops.

bf16 data path (PSUM accumulation stays fp32): avoids the fp32 HI/LO matmul
split, enables FWL weight loads and DVE 2x. All per-core inputs travel in ONE
[128, 2736] "mega" tensor so a single DMA semaphore covers every constant.
"""
import os
import numpy as np
import ml_dtypes

import concourse.bacc as bacc
import concourse.bass as bass
import concourse.mybir as mybir
import concourse.tile as tile
from concourse.bass_utils import run_bass_kernel_spmd

F32 = mybir.dt.float32
BF16 = mybir.dt.bfloat16
NPBF = ml_dtypes.bfloat16

B, C, H, W = 2, 32, 64, 128
D, DP = 48, 192
H4, W4 = 256, 512
HB = 64            # h' rows per core
HS = 18            # source h rows needed
HPAD = 20          # padded so C*HPAD = 640 = 5*128
KCH = 5            # K chunks of 128 for the (c,h) contraction
H_START = [0, 15, 31, 47]

# mega layout (free-dim offsets, bf16 elements)
OFF_LP, OFF_AT, OFF_RP = 0, 640, 960
OFF_MASK, OFF_V = 1600, 1648
OFF_UBLK, OFF_OD4 = 2160, 2544
MEGA_F = 2556


def _interp_pairs(in_size, out_size):
    src = (np.arange(out_size, dtype=np.float32)
           * np.float32((in_size - 1) / (out_size - 1)))
    i0 = np.clip(np.floor(src).astype(np.int32), 0, in_size - 1)
    i1 = np.clip(i0 + 1, 0, in_size - 1)
    w = (src - i0.astype(np.float32)).astype(np.float32)
    return i0, i1, w


def _interp_matrix(in_size, out_size):
    i0, i1, w = _interp_pairs(in_size, out_size)
    M = np.zeros((in_size, out_size), dtype=np.float32)
    for o in range(out_size):
        M[i0[o], o] += np.float32(1.0) - w[o]
        M[i1[o], o] += w[o]
    return M


def _shared_mega():
    """The core-independent part of the mega input."""
    mega = np.zeros((128, MEGA_F), np.float32)
    mega[:, OFF_MASK:OFF_MASK + D] = (
        np.arange(W)[:, None] >= (D - 1 - np.arange(D))[None, :]).astype(np.float32)
    mega[:, OFF_V:OFF_V + W4] = _interp_matrix(W, W4)
    U_rev = _interp_matrix(D, DP)[::-1]
    # block-diag U over an h'-pair: rows (s, dr), cols (s, d') -> 3 M-chunks
    mega[0:48, OFF_UBLK:OFF_UBLK + DP] = U_rev
    mega[48:96, OFF_UBLK + DP:OFF_UBLK + 2 * DP] = U_rev
    # od4[k]: [Z_s0 | Z_s1 | N_s0 | N_s1] selectors for chunk-k's 128 rows
    od4 = np.zeros((128, 3, 4), np.float32)
    od4[:, 0, 0] = 1.0
    od4[:, 0, 2] = np.arange(128)
    od4[0:64, 1, 0] = 1.0
    od4[0:64, 1, 2] = np.arange(128, DP)
    od4[64:128, 1, 1] = 1.0
    od4[64:128, 1, 3] = np.arange(0, 64)
    od4[:, 2, 1] = 1.0
    od4[:, 2, 3] = np.arange(64, DP)
    mega[:, OFF_OD4:OFF_OD4 + 12] = od4.reshape(128, 12)
    return mega


def _core_mega(shared, left, right, b, j):
    hs = H_START[j]
    nvalid = min(H, hs + HS) - hs
    lp = np.zeros((C, HPAD, W), np.float32)
    rp = np.zeros((C, HPAD, W), np.float32)
    lp[:, :nvalid] = left[b, :, hs:hs + nvalid]
    rp[:, :nvalid] = right[b, :, hs:hs + nvalid]

    i0, i1, w = _interp_pairs(H, H4)
    A = np.zeros((HB, HPAD), np.float32)
    inv2c = np.float32(1.0 / (2 * C))
    for i in range(HB):
        hp = HB * j + i
        A[i, i0[hp] - hs] += (np.float32(1.0) - w[hp]) * inv2c
        A[i, i1[hp] - hs] += w[hp] * inv2c
    aT = np.ascontiguousarray(
        np.broadcast_to(A.T[None], (C, HPAD, HB))).reshape(KCH, 128, HB)

    mega = shared.copy()
    # [(k p), x] -> [p, (k x)]
    mega[:, OFF_LP:OFF_LP + 640] = (
        lp.reshape(KCH, 128, W).transpose(1, 0, 2).reshape(128, KCH * W))
    mega[:, OFF_RP:OFF_RP + 640] = (
        rp.reshape(KCH, 128, W).transpose(1, 0, 2).reshape(128, KCH * W))
    mega[:, OFF_AT:OFF_AT + 320] = (
        aT.transpose(1, 0, 2).reshape(128, KCH * HB))
    return mega.astype(NPBF)


def build_nc():
    nc = bacc.Bacc("TRN2", target_bir_lowering=False, debug=False)

    mega_d = nc.declare_dram_parameter("mega", [128, MEGA_F], BF16, isOutput=False)
    outt_d = nc.declare_dram_parameter("outt", [HB, W4], F32, isOutput=True)
    srhp_dram = nc.dram_tensor("srhp", [HB, D + W], BF16)  # zero-padded SRH
    zd_dram = nc.dram_tensor("zd", [2, 16384], F32)        # Z|N reshape staging

    EXP = mybir.ActivationFunctionType.Exp
    AX = mybir.AxisListType.X

    with tile.TileContext(nc) as tc:
        with tc.tile_pool(name="consts", bufs=1) as cpool:
            mega_sb = cpool.tile([128, MEGA_F], BF16)
            nc.sync.dma_start(mega_sb[:], mega_d[:])

            lp_v = mega_sb[:, OFF_LP:OFF_LP + 640].rearrange("p (k w) -> p k w", k=KCH)
            rp_v = mega_sb[:, OFF_RP:OFF_RP + 640].rearrange("p (k w) -> p k w", k=KCH)
            aT_v = mega_sb[:, OFF_AT:OFF_AT + 320].rearrange("p (k m) -> p k m", k=KCH)
            mask_v = mega_sb[:, OFF_MASK:OFF_MASK + D]
            v_v = mega_sb[:, OFF_V:OFF_V + W4]
            ublk_v = mega_sb[0:96, OFF_UBLK:OFF_UBLK + 2 * DP]
            od4_v = mega_sb[:, OFF_OD4:OFF_OD4 + 12].rearrange(
                "p (k f) -> p k f", k=3)

            # Stage A: SLH^T = lp^T @ aT  (contract (c,h)),  SRH = aT^T @ rp
            slht_sb = cpool.tile([W, HB], BF16)
            srhp_sb = cpool.tile([HB, D + W], BF16)
            with tc.tile_pool(name="psA", bufs=1, space="PSUM") as psA:
                slht_ps = psA.tile([W, HB], F32)
                srh_ps = psA.tile([HB, W], F32)
                for k in range(KCH):
                    nc.tensor.matmul(slht_ps[:], lp_v[:, k, :], aT_v[:, k, :],
                                     start=(k == 0), stop=(k == KCH - 1))
                for k in range(KCH):
                    nc.tensor.matmul(srh_ps[:], aT_v[:, k, :], rp_v[:, k, :],
                                     start=(k == 0), stop=(k == KCH - 1))

                nc.vector.tensor_copy(slht_sb[:], slht_ps[:])
                nc.vector.memset(srhp_sb[:, 0:D], 0.0)
                nc.vector.tensor_copy(srhp_sb[:, D:D + W], srh_ps[:])
            nc.sync.dma_start(srhp_dram[:], srhp_sb[:])

            # m2[w, h', dr] = maskT[w,dr] * (SLH^T[w,h'] + SRH_pad[h', w+dr+1])
            g_sb = cpool.tile([96, 32, W4], BF16)  # [(s,dr), pair, w']
            with tc.tile_pool(name="mwork", bufs=1) as mpool:
                toep_sb = mpool.tile([W, HB, D], BF16)
                toep_src = bass.AP(srhp_dram, 1, [[1, W], [D + W, HB], [1, D]])
                nc.sync.dma_start(toep_sb[:], toep_src)

                m_sb = mpool.tile([W, HB, D], BF16)
                slht_b = slht_sb[:].unsqueeze(2).broadcast_to((W, HB, D))
                nc.vector.tensor_add(m_sb[:], toep_sb[:], slht_b)
                m2_sb = mpool.tile([W, HB, D], BF16)
                mask_b = mask_v.unsqueeze(1).broadcast_to((W, HB, D))
                nc.vector.tensor_mul(m2_sb[:], m_sb[:], mask_b)

                # W-upsample: g[(s,dr), pair, w'] = sum_w m2[w, 2p+s, dr] V[w,w']
                with tc.tile_pool(name="psG", bufs=3, space="PSUM") as psG:
                    for p in range(32):
                        g_ps = psG.tile([96, W4], F32)
                        nc.tensor.matmul(g_ps[:], m2_sb[:, 2 * p:2 * p + 2, :],
                                         v_v, start=True, stop=True)
                        nc.vector.tensor_copy(g_sb[:, p, :], g_ps[:])

            # D-up -> f [(s,d') 3x128-chunks, w'] per pair, exp, then Z|N
            # via 4-column selector matmuls; divide on dense repacked tiles.
            with (
                tc.tile_pool(name="epool", bufs=1) as epool,
                tc.tile_pool(name="spool", bufs=2) as spool,
            ):
                for sb in range(2):  # superblocks of 16 pairs (32 h')
                    e_sb = epool.tile([128, 16, 3, W4], BF16, tag="e")
                    with tc.tile_pool(name="psF", bufs=2, space="PSUM") as psF:
                        for t in range(16):
                            pair = 16 * sb + t
                            f_ps = psF.tile([128, 3, W4], F32, tag="f")
                            for k in range(3):
                                nc.tensor.matmul(
                                    f_ps[:, k, :],
                                    ublk_v[:, 128 * k:128 * (k + 1)],
                                    g_sb[:, pair, :], start=True, stop=True)
                            nc.scalar.activation(e_sb[:, t, :, :], f_ps[:], EXP)

                    # Z/N: out [4=(Z_s0,N_s0,Z_s1,N_s1), 512] per pair
                    with tc.tile_pool(name="psZ", bufs=2, space="PSUM") as psZ:
                        for gt in range(4):  # 4 pairs per PSUM tile
                            znb = psZ.tile([4, 4, W4], F32, tag="zn")
                            for j in range(4):
                                t = 4 * gt + j
                                for k in range(3):
                                    nc.tensor.matmul(
                                        znb[:, j, :], od4_v[:, k, :],
                                        e_sb[:, t, k, :], start=(k == 0),
                                        stop=(k == 2))
                            zsb_t = spool.tile([4, 4, W4], F32, tag="zsb")
                            nc.vector.tensor_copy(zsb_t[:], znb[:])
                            # rows (Zs0,Zs1 | Ns0,Ns1) -> pixel-ordered planes
                            # zd plane addr: hl*512 + w', hl = 8gt + 2ps + s
                            for pl in range(2):
                                nc.sync.dma_start(
                                    bass.AP(zd_dram, 16384 * pl + 4096 * gt,
                                            [[512, 2], [1024, 4], [1, W4]]),
                                    zsb_t[2 * pl:2 * pl + 2, :, :])
                    zc_t = spool.tile([128, 2, 128], F32, tag="zc")
                    nc.sync.dma_start(
                        zc_t[:], bass.AP(zd_dram, 0,
                                         [[1, 128], [16384, 2], [128, 128]]))
                    rz_t = spool.tile([128, 128], F32, tag="rz")
                    nc.vector.reciprocal(rz_t[:], zc_t[:, 0, :])
                    oc_t = spool.tile([128, 128], F32, tag="oc")
                    nc.vector.tensor_mul(oc_t[:], zc_t[:, 1, :], rz_t[:])
                    nc.sync.dma_start(
                        bass.AP(outt_d, 16384 * sb, [[1, 128], [128, 128]]),
                        oc_t[:])
    nc.compile()
    return nc


_NC = None


def _in_maps(left, right):
    shared = _shared_mega()
    return [{"mega": _core_mega(shared, left, right, k // 4, k % 4)}
            for k in range(8)]


def kernel(left, right):
    global _NC
    left = np.asarray(left, dtype=np.float32)
    right = np.asarray(right, dtype=np.float32)
    if _NC is None:
        _NC = build_nc()

    res = run_bass_kernel_spmd(_NC, _in_maps(left, right), core_ids=list(range(8)))
    out = np.zeros((B, H4, W4), np.float32)
    for k in range(8):
        b, j = k // 4, k % 4
        out[b, HB * j:HB * (j + 1)] = res.results[k]["outt"]
    return out
